# revision 3
# baseline (speedup 1.0000x reference)
"""MiniChessNNUE kernel for 8 Trainium2 NeuronCores.

Data-parallel: batch (16384) sharded 2048/core, weights replicated.

Math (per core, batch slice n):
  w_acc = screlu(white @ ft_w.T + ft_b)      [n, 128]
  b_acc = screlu(black @ ft_w.T + ft_b)      [n, 128]
  x     = concat(where(stm, b_acc, w_acc), where(stm, w_acc, b_acc))
  z1    = x @ l1_w.T + l1_b
With l1_w = [A | B] the select folds into matmul algebra:
  z1 = A@w' + B@b' + (A-B)@(stm * (b'-w'))   (w'/b' = screlu'd accums)
Everything on device is computed transposed ([feature, batch] layout) so the
contraction dim sits on SBUF partitions and every bias becomes a K=1 rank-1
matmul against a ones row.

The batch is processed in STAGES of n-chunks so each stage's small-MLP tail
hides under the next stage's feature streaming (the kernel is memory-bound on
the two big feature matrices).
"""

import os

import numpy as np
import ml_dtypes

import concourse.bass as bass  # noqa: F401
import concourse.tile as tile
from concourse import bacc, mybir
from concourse.bass_utils import run_bass_kernel_spmd

# Containers without the full antenv package lack the axon NTFF hook module
# that run_bass_kernel_spmd imports when BASS_TRACE is set; stub it so trace
# requests degrade to "no trace" instead of crashing.
try:
    from antenv import axon_hooks as _axon_hooks  # noqa: F401
except ImportError:
    import sys
    import types

    _m = types.ModuleType("antenv.axon_hooks")
    _m.get_axon_ntff_profile_hook = lambda: None
    sys.modules["antenv.axon_hooks"] = _m

N_CORES = 8
B = 16384
F = 9000
ACC = 128
L1 = 32
L2 = 32

BC = B // N_CORES        # 2048 batch rows per core
KP = 125                 # contraction partitions per chunk (9000 = 72 * 125)
NK = F // KP             # 72 k-chunks
NFREE = 512              # matmul moving free dim (one PSUM bank of fp32)
NCH = BC // NFREE        # 4 n-chunks per core

# f32 = exact fp32 everywhere; bf16/f16 = features/ft_w/MLP cast to 16-bit
# (half the HBM traffic, full PE rate; f16 has 8x the mantissa of bf16 and
# every tensor here lives in [-1, 1])
FEAT_MODE = os.environ.get("NNUE_FEAT_MODE", "f16")
KO = int(os.environ.get("NNUE_KO", "8"))          # k-chunks per DMA tile
STAGES = int(os.environ.get("NNUE_STAGES", "4"))  # batch pipeline stages
FEAT_BUFS = int(os.environ.get("NNUE_FEAT_BUFS", "6"))

F32 = mybir.dt.float32

LAST_RESULT = None  # BassKernelResults of the most recent run (for profiling)


def _build(feat_mode: str, ko: int = KO, feat_bufs: int = FEAT_BUFS,
           stages: int = STAGES, reps: int = 1):
    fdt = {"bf16": mybir.dt.bfloat16, "f16": mybir.dt.float16,
           "f32": F32}[feat_mode]
    # MLP precision follows the feature precision (fp32 matmuls are 4x
    # slower on the PE, and in f32 mode we are PE-bound anyway).
    mdt = fdt
    nt = NK // ko
    nch_s = NCH // stages          # n-chunks per stage
    cols = nch_s * NFREE           # batch columns per stage

    nc = bacc.Bacc("TRN2", target_bir_lowering=False, debug=False)
    # features are host pre-tiled into the exact DMA order: tile i = st*nt+t
    # is one contiguous [KP, ko, cols] block (max-efficiency HBM reads)
    wT = nc.dram_tensor("wT", [stages * nt, KP, ko, cols], fdt,
                        kind="ExternalInput")
    bT = nc.dram_tensor("bT", [stages * nt, KP, ko, cols], fdt,
                        kind="ExternalInput")
    # host pre-permuted so partition p holds rows {k*125+p} contiguously
    ftwT = nc.dram_tensor("ftwT", [KP, NK, ACC], fdt, kind="ExternalInput")
    ftb = nc.dram_tensor("ftb", [1, ACC], F32, kind="ExternalInput")
    smask = nc.dram_tensor("smask", [ACC, BC], mdt, kind="ExternalInput")
    l1A = nc.dram_tensor("l1A", [ACC, L1], mdt, kind="ExternalInput")
    l1B = nc.dram_tensor("l1B", [ACC, L1], mdt, kind="ExternalInput")
    l1D = nc.dram_tensor("l1D", [ACC, L1], mdt, kind="ExternalInput")
    l1b = nc.dram_tensor("l1b", [1, L1], mdt, kind="ExternalInput")
    l2wT = nc.dram_tensor("l2wT", [L1, L2], mdt, kind="ExternalInput")
    l2b = nc.dram_tensor("l2b", [1, L2], mdt, kind="ExternalInput")
    owT = nc.dram_tensor("owT", [L2, 1], mdt, kind="ExternalInput")
    ob = nc.dram_tensor("ob", [1, 1], mdt, kind="ExternalInput")
    y = nc.dram_tensor("y", [1, BC], F32, kind="ExternalOutput")

    wT_r = wT[:]
    bT_r = bT[:]

    with tile.TileContext(nc) as tc:
        with (
            tc.tile_pool(name="consts", bufs=1) as consts,
            tc.tile_pool(name="feat", bufs=feat_bufs) as featp,
            tc.tile_pool(name="acts", bufs=2) as actp,
            tc.tile_pool(name="psum_ft", bufs=4, space="PSUM") as psum_ft,
            tc.tile_pool(name="psum_s", bufs=3, space="PSUM") as psum_s,
        ):
            ftw_sb = consts.tile([KP, NK, ACC], fdt)
            nc.sync.dma_start(ftw_sb[:], ftwT[:])
            ones_sb = consts.tile([1, NFREE], F32)
            nc.vector.memset(ones_sb[:], 1.0)
            ones_m = ones_sb
            if mdt != F32:
                ones_m = consts.tile([1, NFREE], mdt)
                nc.vector.memset(ones_m[:], 1.0)
            ftb_sb = consts.tile([1, ACC], F32)
            nc.sync.dma_start(ftb_sb[:], ftb[:])
            smask_sb = consts.tile([ACC, BC], mdt)
            nc.scalar.dma_start(smask_sb[:], smask[:])
            l1A_sb = consts.tile([ACC, L1], mdt)
            nc.scalar.dma_start(l1A_sb[:], l1A[:])
            l1B_sb = consts.tile([ACC, L1], mdt)
            nc.scalar.dma_start(l1B_sb[:], l1B[:])
            l1D_sb = consts.tile([ACC, L1], mdt)
            nc.scalar.dma_start(l1D_sb[:], l1D[:])
            l1b_sb = consts.tile([1, L1], mdt)
            nc.scalar.dma_start(l1b_sb[:], l1b[:])
            l2wT_sb = consts.tile([L1, L2], mdt)
            nc.scalar.dma_start(l2wT_sb[:], l2wT[:])
            l2b_sb = consts.tile([1, L2], mdt)
            nc.scalar.dma_start(l2b_sb[:], l2b[:])
            owT_sb = consts.tile([L2, 1], mdt)
            nc.scalar.dma_start(owT_sb[:], owT[:])
            ob_sb = consts.tile([1, 1], mdt)
            nc.scalar.dma_start(ob_sb[:], ob[:])

            y_sb = consts.tile([1, BC], F32)

            # W stream on the SP HWDGE ring, B stream on the ACT ring: the
            # per-DMA fixed cost serializes per ring, so split across both.
            dma_eng = (nc.sync, nc.scalar)

            import contextlib
            rep_ctx = tc.For_i(0, reps, 1) if reps > 1 else contextlib.nullcontext()
            with rep_ctx:
                _emit_body(nc, tc, stages, ko, feat_bufs, fdt, mdt,
                           locals())

            nc.sync.dma_start(y[:], y_sb[:])

    nc.compile()
    return nc


def _emit_body(nc, tc, stages, ko, feat_bufs, fdt, mdt, env):
    (wT_r, bT_r, ftw_sb, ones_sb, ones_m, ftb_sb, smask_sb, l1A_sb, l1B_sb,
     l1D_sb, l1b_sb, l2wT_sb, l2b_sb, owT_sb, ob_sb, y_sb, featp, actp,
     psum_ft, psum_s, dma_eng) = (
        env["wT_r"], env["bT_r"], env["ftw_sb"], env["ones_sb"], env["ones_m"],
        env["ftb_sb"], env["smask_sb"], env["l1A_sb"], env["l1B_sb"],
        env["l1D_sb"], env["l1b_sb"], env["l2wT_sb"], env["l2b_sb"],
        env["owT_sb"], env["ob_sb"], env["y_sb"], env["featp"], env["actp"],
        env["psum_ft"], env["psum_s"], env["dma_eng"])
    nt = NK // ko
    nch_s = NCH // stages
    cols = nch_s * NFREE
    if True:
        for st in range(stages):
                c0 = st * cols
                # ---- feature transformer, one perspective at a time ----
                sq = []  # screlu'd accumulators, transposed: [128, cols]
                for pi, src in enumerate((wT_r, bT_r)):
                    acc_ps = [psum_ft.tile([ACC, NFREE], F32, tag="acc",
                                           name=f"acc_{st}_{pi}_{n}")
                              for n in range(nch_s)]
                    # bias as rank-1 update opens each accumulation group
                    for n in range(nch_s):
                        nc.tensor.matmul(acc_ps[n][:], ftb_sb[:], ones_sb[:],
                                         start=True, stop=False)
                    for t in range(nt):
                        ft_tile = featp.tile([KP, ko, cols], fdt, tag="feat",
                                             name=f"ft_{st}_{pi}_{t}")
                        dma_eng[pi].dma_start(ft_tile[:], src[st * nt + t])
                        for kk in range(ko):
                            k = t * ko + kk
                            for n in range(nch_s):
                                nc.tensor.matmul(
                                    acc_ps[n][:],
                                    ftw_sb[:, k, :],
                                    ft_tile[:, kk, n * NFREE:(n + 1) * NFREE],
                                    start=False,
                                    stop=(k == NK - 1),
                                )
                    sq_sb = actp.tile([ACC, cols], mdt, tag=f"sq{pi}",
                                      name=f"sq_{st}_{pi}")
                    for n in range(nch_s):
                        s = sq_sb[:, n * NFREE:(n + 1) * NFREE]
                        # screlu: clamp to [0, 1] then square
                        nc.vector.tensor_scalar(
                            s, acc_ps[n][:], 0.0, 1.0,
                            mybir.AluOpType.max, mybir.AluOpType.min)
                        nc.vector.tensor_mul(out=s, in0=s, in1=s)
                    sq.append(sq_sb)

                # d = stm * (b' - w')
                d_sb = actp.tile([ACC, cols], mdt, tag="d", name=f"d_{st}")
                nc.vector.tensor_sub(out=d_sb[:], in0=sq[1][:], in1=sq[0][:])
                nc.vector.tensor_mul(out=d_sb[:], in0=d_sb[:],
                                     in1=smask_sb[:, c0:c0 + cols])

                h1_sb = actp.tile([L1, cols], mdt, tag="h1", name=f"h1_{st}")
                h2_sb = actp.tile([L2, cols], mdt, tag="h2", name=f"h2_{st}")
                for n in range(nch_s):
                    ns = slice(n * NFREE, (n + 1) * NFREE)
                    p1 = psum_s.tile([L1, NFREE], F32, tag="ps_s",
                                     name=f"p1_{st}_{n}")
                    nc.tensor.matmul(p1[:], l1A_sb[:], sq[0][:, ns], start=True, stop=False)
                    nc.tensor.matmul(p1[:], l1B_sb[:], sq[1][:, ns], start=False, stop=False)
                    nc.tensor.matmul(p1[:], l1D_sb[:], d_sb[:, ns], start=False, stop=False)
                    nc.tensor.matmul(p1[:], l1b_sb[:], ones_m[:], start=False, stop=True)
                    h1 = h1_sb[:, ns]
                    nc.vector.tensor_scalar(h1, p1[:], 0.0, 1.0,
                                            mybir.AluOpType.max, mybir.AluOpType.min)
                    nc.vector.tensor_mul(out=h1, in0=h1, in1=h1)

                    p2 = psum_s.tile([L2, NFREE], F32, tag="ps_s",
                                     name=f"p2_{st}_{n}")
                    nc.tensor.matmul(p2[:], l2wT_sb[:], h1, start=True, stop=False)
                    nc.tensor.matmul(p2[:], l2b_sb[:], ones_m[:], start=False, stop=True)
                    h2 = h2_sb[:, ns]
                    nc.vector.tensor_scalar(h2, p2[:], 0.0, 1.0,
                                            mybir.AluOpType.max, mybir.AluOpType.min)
                    nc.vector.tensor_mul(out=h2, in0=h2, in1=h2)

                    p3 = psum_s.tile([1, NFREE], F32, tag="ps_s",
                                     name=f"p3_{st}_{n}")
                    nc.tensor.matmul(p3[:], owT_sb[:], h2, start=True, stop=False)
                    nc.tensor.matmul(p3[:], ob_sb[:], ones_m[:], start=False, stop=True)
                    nc.vector.tensor_copy(out=y_sb[:, c0 + n * NFREE:c0 + (n + 1) * NFREE],
                                          in_=p3[:])


_NC_CACHE: dict = {}


def _pretile(arr_T, ko=None, stages=None):
    """[F, BC] (transposed features) -> [S*NT, KP, ko, cols] in device DMA
    order, so each feature tile is one contiguous HBM block."""
    ko = KO if ko is None else ko
    stages = STAGES if stages is None else stages
    nt = NK // ko
    cols = (NCH // stages) * NFREE
    return np.ascontiguousarray(
        arr_T.reshape(nt, ko, KP, stages, cols)
             .transpose(3, 0, 2, 1, 4)
             .reshape(stages * nt, KP, ko, cols))


def prepare_in_maps(white_features, black_features, stm, ft_w, ft_b,
                    l1_w, l1_b, l2_w, l2_b, out_w, out_b,
                    feat_mode: str = None) -> list:
    feat_mode = FEAT_MODE if feat_mode is None else feat_mode
    feat_np = {"bf16": ml_dtypes.bfloat16, "f16": np.float16,
               "f32": np.float32}[feat_mode]
    mlp_np = feat_np

    white_features = np.asarray(white_features)
    black_features = np.asarray(black_features)
    stm = np.asarray(stm)
    ft_w = np.asarray(ft_w, dtype=np.float32)
    ft_b = np.asarray(ft_b, dtype=np.float32)
    l1_w = np.asarray(l1_w, dtype=np.float32)
    l1_b = np.asarray(l1_b, dtype=np.float32)
    l2_w = np.asarray(l2_w, dtype=np.float32)
    l2_b = np.asarray(l2_b, dtype=np.float32)
    out_w = np.asarray(out_w, dtype=np.float32)
    out_b = np.asarray(out_b, dtype=np.float32)

    # [F, 128] -> [125, 72, 128] with [p, k, m] = ft_w.T[k*125+p, m], so the
    # device DMA is one fully-contiguous read
    ftwT = np.ascontiguousarray(
        ft_w.T.astype(feat_np).reshape(NK, KP, ACC).transpose(1, 0, 2))
    A = l1_w[:, :ACC]
    Bm = l1_w[:, ACC:]
    shared = {
        "ftwT": ftwT,
        "ftb": np.ascontiguousarray(ft_b[None, :]),              # [1, 128]
        "l1A": np.ascontiguousarray(A.T).astype(mlp_np),         # [128, 32]
        "l1B": np.ascontiguousarray(Bm.T).astype(mlp_np),
        "l1D": np.ascontiguousarray((A - Bm).T).astype(mlp_np),
        "l1b": l1_b[None, :].astype(mlp_np),
        "l2wT": np.ascontiguousarray(l2_w.T).astype(mlp_np),     # [32, 32]
        "l2b": l2_b[None, :].astype(mlp_np),
        "owT": np.ascontiguousarray(out_w.T).astype(mlp_np),     # [32, 1]
        "ob": out_b[None, :].astype(mlp_np),                     # [1, 1]
    }

    stm_f = stm.astype(np.float32)
    in_maps = []
    for c in range(N_CORES):
        sl = slice(c * BC, (c + 1) * BC)
        wc = white_features[sl].astype(feat_np, copy=False)
        bc = black_features[sl].astype(feat_np, copy=False)
        in_maps.append({
            "wT": _pretile(wc.T),
            "bT": _pretile(bc.T),
            "smask": np.ascontiguousarray(
                np.broadcast_to(stm_f[sl][None, :], (ACC, BC))).astype(mlp_np),
            **shared,
        })
    return in_maps


def kernel(white_features, black_features, stm, ft_w, ft_b,
           l1_w, l1_b, l2_w, l2_b, out_w, out_b) -> np.ndarray:
    global LAST_RESULT
    feat_mode = FEAT_MODE
    in_maps = prepare_in_maps(white_features, black_features, stm, ft_w, ft_b,
                              l1_w, l1_b, l2_w, l2_b, out_w, out_b, feat_mode)

    if feat_mode not in _NC_CACHE:
        _NC_CACHE[feat_mode] = _build(feat_mode)
    nc = _NC_CACHE[feat_mode]

    LAST_RESULT = run_bass_kernel_spmd(nc, in_maps, core_ids=list(range(N_CORES)))
    out = np.concatenate(
        [LAST_RESULT.results[c]["y"].reshape(BC) for c in range(N_CORES)])
    return out.astype(np.float32)

